# revision 1
# baseline (speedup 1.0000x reference)
# Involution2d (K=7) Trainium2 kernel — 8-core SPMD, batch+spatial sharding.
#
# Sharding: 8 cores = (batch b in 0..3) x (H-half in 0..1). Each core computes
# a [128, 32, 64] output block. Per core, on device:
#   1. kernel generation: 1x1 conv (BN folded) -> ReLU -> 1x1 conv -> [49, 2048]
#      per-pixel involution kernels (+ bias, x boundary mask folded in).
#   2. involution: acc[c, p] = sum_o kerm[o, p] * xw[c, p + shift_o]
#      - ker rows broadcast across 128 partitions via K=1 matmuls on TensorE
#      - multiply/accumulate on VectorE; row shifts are free-dim AP offsets
#        into a halo-padded x layout; W-edge wraps are killed by the mask.
import numpy as np

EPS = 1e-5
KK = 7
C = 128
H = 64
W = 64
B = 4
HH = 32            # rows per core
P = HH * W         # 2048 output pixels per core
NIN = 4 + 38 * W + 4   # 2440: 3-row halos + 4-elem guard pads each side
GEN_CHUNK = 512
BC_CHUNK = 1024    # broadcast/psum chunk (2 PSUM banks)

_STATE = {}


def _build():
    import concourse.tile as tile
    from concourse import bacc, mybir

    f32 = mybir.dt.float32
    nc = bacc.Bacc("TRN2", target_bir_lowering=False, debug=False)

    xw_d = nc.dram_tensor("xw", [C, NIN], f32, kind="ExternalInput").ap()
    w1sT_d = nc.dram_tensor("w1sT", [C, 32], f32, kind="ExternalInput").ap()
    b1f_d = nc.dram_tensor("b1f", [32, 1], f32, kind="ExternalInput").ap()
    w2T_d = nc.dram_tensor("w2T", [32, 49], f32, kind="ExternalInput").ap()
    b2f_d = nc.dram_tensor("b2f", [49, 1], f32, kind="ExternalInput").ap()
    mask_d = nc.dram_tensor("maskt", [49, P], f32, kind="ExternalInput").ap()
    out_d = nc.dram_tensor("out", [C, P], f32, kind="ExternalOutput").ap()

    with tile.TileContext(nc) as tc:
        with (
            tc.tile_pool(name="consts", bufs=1) as cpool,
            tc.tile_pool(name="work", bufs=2) as wpool,
            tc.tile_pool(name="pgen", bufs=2, space="PSUM") as pgen,
            tc.tile_pool(name="pbc", bufs=2, space="PSUM") as pbc,
        ):
            x_sb = cpool.tile([C, NIN], f32, tag="x")
            nc.sync.dma_start(x_sb[:], xw_d)
            w1sT = cpool.tile([C, 32], f32, tag="w1")
            nc.sync.dma_start(w1sT[:], w1sT_d)
            b1f = cpool.tile([32, 1], f32, tag="b1")
            nc.sync.dma_start(b1f[:], b1f_d)
            w2T = cpool.tile([32, 49], f32, tag="w2")
            nc.sync.dma_start(w2T[:], w2T_d)
            b2f = cpool.tile([49, 1], f32, tag="b2")
            nc.sync.dma_start(b2f[:], b2f_d)
            mask_sb = cpool.tile([49, P], f32, tag="mask")
            nc.sync.dma_start(mask_sb[:], mask_d)
            ones_sb = cpool.tile([1, C], f32, tag="ones")
            nc.vector.memset(ones_sb[:], 1.0)

            f_sb = cpool.tile([32, P], f32, tag="f")
            kerm_sb = cpool.tile([49, P], f32, tag="kerm")
            acc_sb = cpool.tile([C, P], f32, tag="acc")

            # ---- kernel generation ----
            # x view for the core's own rows: starts 3 halo rows in (+4 guard)
            XOFF = 4 + 3 * W
            for ci in range(P // GEN_CHUNK):
                sl = slice(ci * GEN_CHUNK, (ci + 1) * GEN_CHUNK)
                xsl = slice(XOFF + ci * GEN_CHUNK, XOFF + (ci + 1) * GEN_CHUNK)
                f1 = pgen.tile([32, GEN_CHUNK], f32, tag="f1")
                nc.tensor.matmul(f1[:], w1sT[:], x_sb[:, xsl], start=True, stop=True)
                # f = relu(f1 + b1f)  (ScalarE, per-partition bias)
                nc.scalar.activation(
                    f_sb[:, sl], f1[:], mybir.ActivationFunctionType.Relu,
                    bias=b1f[:],
                )
                k2 = pgen.tile([49, GEN_CHUNK], f32, tag="k2")
                nc.tensor.matmul(k2[:], w2T[:], f_sb[:, sl], start=True, stop=True)
                # kerm = (k2 + b2) * mask  (VectorE fused)
                nc.vector.scalar_tensor_tensor(
                    out=kerm_sb[:, sl], in0=k2[:], scalar=b2f[:],
                    in1=mask_sb[:, sl],
                    op0=mybir.AluOpType.add, op1=mybir.AluOpType.mult,
                )

            # ---- involution accumulate ----
            NB = BC_CHUNK // 512
            for o in range(49):
                ip, jp = divmod(o, 7)
                A = W * ip + jp + 1
                # matmul rhs must start at partition 0 -> DMA ker row o there
                krow = wpool.tile([1, P], f32, tag="krow")
                nc.sync.dma_start(krow[:], kerm_sb[o:o + 1, :])
                for h2 in range(P // BC_CHUNK):
                    bc = pbc.tile([C, BC_CHUNK], f32, tag="bc")
                    base = h2 * BC_CHUNK
                    for nb in range(NB):
                        nc.tensor.matmul(
                            bc[:, nb * 512:(nb + 1) * 512],
                            ones_sb[:],
                            krow[0:1, base + nb * 512: base + (nb + 1) * 512],
                            start=True, stop=True,
                        )
                    xs = x_sb[:, A + base: A + base + BC_CHUNK]
                    osl = slice(base, base + BC_CHUNK)
                    if o == 0:
                        nc.vector.tensor_mul(acc_sb[:, osl], xs, bc[:])
                    else:
                        prod = wpool.tile([C, BC_CHUNK], f32, tag="prod")
                        nc.vector.tensor_mul(prod[:], xs, bc[:])
                        nc.vector.tensor_add(acc_sb[:, osl], acc_sb[:, osl], prod[:])

            nc.sync.dma_start(out_d, acc_sb[:])

    nc.compile()
    return nc


def _get_nc():
    if "nc" not in _STATE:
        _STATE["nc"] = _build()
    return _STATE["nc"]


def _host_prep(x, w1, b1, bn_gamma, bn_beta, bn_mean, bn_var, w2, b2):
    x = np.asarray(x, dtype=np.float32)
    scale = np.asarray(bn_gamma) / np.sqrt(np.asarray(bn_var) + EPS)
    w1s = (np.asarray(w1) * scale[:, None]).astype(np.float32)
    b1f = (np.asarray(b1) * scale + np.asarray(bn_beta)
           - np.asarray(bn_mean) * scale).astype(np.float32)
    w1sT = np.ascontiguousarray(w1s.T)                      # [128, 32]
    w2T = np.ascontiguousarray(np.asarray(w2, np.float32).T)  # [32, 49]
    b1fc = np.ascontiguousarray(b1f[:, None])               # [32, 1]
    b2fc = np.ascontiguousarray(np.asarray(b2, np.float32)[:, None])  # [49, 1]

    # W-edge mask: kerm[o, p] = 0 where w + dj leaves the row
    wcol = np.arange(P, dtype=np.int64) % W
    maskt = np.zeros((49, P), dtype=np.float32)
    for ipp in range(KK):
        for jpp in range(KK):
            dj = jpp - 3
            maskt[ipp * KK + jpp] = ((wcol + dj >= 0) & (wcol + dj < W))
    maskt = np.ascontiguousarray(maskt)

    in_maps = []
    for core in range(8):
        b, half = divmod(core, 2)
        h0 = HH * half
        xw = np.zeros((C, NIN), dtype=np.float32)
        lo = max(0, h0 - 3)
        hi = min(H, h0 + HH + 3)
        # rows [lo, hi) -> xw positions 4 + 64*(row - h0 + 3)
        src = x[b, :, lo:hi, :].reshape(C, -1)
        start = 4 + W * (lo - h0 + 3)
        xw[:, start:start + src.shape[1]] = src
        in_maps.append({
            "xw": xw, "w1sT": w1sT, "b1f": b1fc, "w2T": w2T,
            "b2f": b2fc, "maskt": maskt,
        })
    return in_maps


def run(inputs: dict, trace: bool = False):
    from concourse.bass_utils import run_bass_kernel_spmd

    nc = _get_nc()
    in_maps = _host_prep(**inputs)
    res = run_bass_kernel_spmd(
        nc, in_maps, core_ids=list(range(8)), trace=trace,
    )
    out = np.zeros((B, C, H, W), dtype=np.float32)
    for core in range(8):
        b, half = divmod(core, 2)
        h0 = HH * half
        out[b, :, h0:h0 + HH, :] = res.results[core]["out"].reshape(C, HH, W)
    return out, res


def kernel(**inputs) -> np.ndarray:
    out, _ = run(inputs, trace=False)
    return out



# revision 3
# speedup vs baseline: 7.2310x; 7.2310x over previous
# Involution2d (K=7) Trainium2 kernel — 8-core SPMD, batch+spatial sharding.
#
# Sharding: 8 cores = (batch b in 0..3) x (H-half in 0..1); each core owns a
# [128, 32, 64] output block (2048 pixels, p = 64*h + w).
#
# Per-core algorithm (all-TensorE involution via a banded pixel->pixel matrix):
#   1. gen (bf16): 1x1 conv (BN folded) -> ReLU -> 1x1 conv, emitted directly
#      in pixel-major layout kermT[p, o] (16 matmuls of [33,128]^T @ [33,49];
#      bias rides an ones-row in the stationary operand).
#   2. GPSIMD local_scatter per 128-pixel tile: place the 49 kernel values of
#      each pixel p at column q - 128*mb of A2T[p, :], where q = p + 192 +
#      64*di + dj is the flattened source pixel. W-edge terms get idx=-1
#      (skipped), which provably clips the window to 512 columns (4 q-tiles).
#   3. TensorE transposes each 128x128 block -> A2[q, p] (bf16, 1 cyc/row).
#   4. involution = 4 accumulating bf16 matmuls per tile:
#      out[c, p] = sum_q xT[q, c] * A2[q, p]   (q-space = 38 rows x 64 = 2432,
#      3 halo rows above/below supplied by the host, zeros at image edges).
import numpy as np
import ml_dtypes

EPS = 1e-5
KK = 7
C = 128
H = 64
W = 64
B = 4
HH = 32            # rows per core
P = HH * W         # 2048 output pixels per core
NQT = 19           # q tiles: (HH + 6) * W / 128
NO = 50            # offset count padded to even (49 + 1 dummy)
AWIN = 512         # scatter window (4 q-tiles)

_STATE = {}

BF16 = ml_dtypes.bfloat16


def _build():
    import concourse.tile as tile
    from concourse import bacc, mybir

    f32 = mybir.dt.float32
    bf16 = mybir.dt.bfloat16
    i16 = mybir.dt.int16
    nc = bacc.Bacc("TRN2", target_bir_lowering=False, debug=False)

    xcm_d = nc.dram_tensor("xcm", [C, P], bf16, kind="ExternalInput").ap()
    xtp_d = nc.dram_tensor("xtp", [128, NQT * 128], bf16, kind="ExternalInput").ap()
    w1sT_d = nc.dram_tensor("w1sT", [C, 32], bf16, kind="ExternalInput").ap()
    b1f_d = nc.dram_tensor("b1f", [32, 1], f32, kind="ExternalInput").ap()
    w2b_d = nc.dram_tensor("w2b", [33, 49], bf16, kind="ExternalInput").ap()
    idx_d = nc.dram_tensor("idxt", [128, NO], i16, kind="ExternalInput").ap()
    ident_d = nc.dram_tensor("ident", [128, 128], bf16, kind="ExternalInput").ap()
    out_d = nc.dram_tensor("out", [C, P], bf16, kind="ExternalOutput").ap()

    with tile.TileContext(nc) as tc:
        with (
            tc.tile_pool(name="consts", bufs=1) as cpool,
            tc.tile_pool(name="a2tp", bufs=1) as a2tp,
            tc.tile_pool(name="blkp", bufs=8) as blkp,
            tc.tile_pool(name="pgen", bufs=2, space="PSUM") as pgen,
            tc.tile_pool(name="ptp", bufs=2, space="PSUM") as ptp,
            tc.tile_pool(name="pout", bufs=2, space="PSUM") as pout,
        ):
            xcm = cpool.tile([C, P], bf16, tag="xcm")
            nc.sync.dma_start(xcm[:], xcm_d)
            xtp = cpool.tile([128, NQT * 128], bf16, tag="xtp")
            nc.sync.dma_start(xtp[:], xtp_d)
            w1sT = cpool.tile([C, 32], bf16, tag="w1")
            nc.sync.dma_start(w1sT[:], w1sT_d)
            b1f = cpool.tile([32, 1], f32, tag="b1")
            nc.sync.dma_start(b1f[:], b1f_d)
            w2b = cpool.tile([33, 49], bf16, tag="w2")
            nc.sync.dma_start(w2b[:], w2b_d)
            idxt = cpool.tile([128, NO], i16, tag="idx")
            nc.sync.dma_start(idxt[:], idx_d)
            ident = cpool.tile([128, 128], bf16, tag="id")
            nc.sync.dma_start(ident[:], ident_d)

            fb = cpool.tile([33, P], bf16, tag="fb")
            kermT = cpool.tile([128, 16 * NO], bf16, tag="kermT")
            outsb = cpool.tile([C, P], bf16, tag="outsb")

            nc.vector.memset(fb[32:33, :], 1.0)

            # ---- kernel generation (pixel-major) ----
            GEN_CHUNK = 512
            for ci in range(P // GEN_CHUNK):
                sl = slice(ci * GEN_CHUNK, (ci + 1) * GEN_CHUNK)
                f1 = pgen.tile([32, GEN_CHUNK], f32, tag="f1")
                nc.tensor.matmul(f1[:], w1sT[:], xcm[:, sl], start=True, stop=True)
                nc.scalar.activation(
                    fb[0:32, sl], f1[:], mybir.ActivationFunctionType.Relu,
                    bias=b1f[:],
                )
            for t in range(16):
                kt = pgen.tile([128, 49], f32, tag="kt")
                nc.tensor.matmul(
                    kt[:], fb[:, 128 * t:128 * (t + 1)], w2b[:],
                    start=True, stop=True,
                )
                if t % 2 == 0:
                    nc.vector.tensor_copy(kermT[:, t * NO:t * NO + 49], kt[:])
                else:
                    nc.scalar.copy(kermT[:, t * NO:t * NO + 49], kt[:])

            # ---- banded-matrix build (GPSIMD scatter, all 16 tiles) ----
            a2ts = []
            for mb in range(16):
                a2t = a2tp.tile([128, AWIN], bf16, tag=f"a2t{mb}")
                nc.gpsimd.local_scatter(
                    a2t[:], kermT[:, mb * NO:(mb + 1) * NO], idxt[:],
                    channels=128, num_elems=AWIN, num_idxs=NO,
                )
                a2ts.append(a2t)

            # ---- transpose blocks + involution matmuls ----
            for mb in range(16):
                blks = []
                for j in range(4):
                    tp = ptp.tile([128, 128], bf16, tag="tp")
                    nc.tensor.transpose(
                        tp[:], a2ts[mb][:, 128 * j:128 * (j + 1)], ident[:]
                    )
                    blk = blkp.tile([128, 128], bf16, tag="blk")
                    if j % 2 == 0:
                        nc.vector.tensor_copy(blk[:], tp[:])
                    else:
                        nc.scalar.copy(blk[:], tp[:])
                    blks.append(blk)
                po = pout.tile([C, 128], f32, tag="po")
                for j in range(4):
                    kb = mb + j
                    nc.tensor.matmul(
                        po[:], xtp[:, kb * 128:(kb + 1) * 128], blks[j][:],
                        start=(j == 0), stop=(j == 3),
                    )
                osl = slice(mb * 128, (mb + 1) * 128)
                if mb % 2 == 0:
                    nc.vector.tensor_copy(outsb[:, osl], po[:])
                else:
                    nc.scalar.copy(outsb[:, osl], po[:])

            nc.sync.dma_start(out_d, outsb[:])

    nc.compile()
    return nc


def _get_nc():
    if "nc" not in _STATE:
        _STATE["nc"] = _build()
    return _STATE["nc"]


def _make_idx_table():
    p_loc = np.arange(128)[:, None]
    o = np.arange(49)[None, :]
    di = o // 7 - 3
    dj = o % 7 - 3
    w_of = p_loc % 64
    idx = p_loc + 192 + 64 * di + dj
    masked = (w_of + dj < 0) | (w_of + dj >= 64)
    idx = np.where(masked, -1, idx)
    tab = np.full((128, NO), -1, dtype=np.int16)
    tab[:, :49] = idx.astype(np.int16)
    return tab


def _host_prep(x, w1, b1, bn_gamma, bn_beta, bn_mean, bn_var, w2, b2):
    x = np.asarray(x, dtype=np.float32)
    scale = np.asarray(bn_gamma) / np.sqrt(np.asarray(bn_var) + EPS)
    w1s = (np.asarray(w1) * scale[:, None]).astype(np.float32)
    b1f = (np.asarray(b1) * scale + np.asarray(bn_beta)
           - np.asarray(bn_mean) * scale).astype(np.float32)
    w1sT = np.ascontiguousarray(w1s.T).astype(BF16)            # [128, 32]
    w2b = np.vstack([np.asarray(w2, np.float32).T,
                     np.asarray(b2, np.float32)[None, :]]).astype(BF16)  # [33, 49]
    b1fc = np.ascontiguousarray(b1f[:, None])                  # [32, 1] f32
    idxt = _make_idx_table()                                   # [128, 50] i16
    ident = np.eye(128, dtype=np.float32).astype(BF16)

    in_maps = []
    for core in range(8):
        b, half = divmod(core, 2)
        h0 = HH * half
        xcm = np.ascontiguousarray(
            x[b, :, h0:h0 + HH, :].reshape(C, P)).astype(BF16)
        # q-space: rows h0-3 .. h0+35 (zeros outside the image)
        xe = np.zeros((C, HH + 6, W), dtype=np.float32)
        lo = max(0, h0 - 3)
        hi = min(H, h0 + HH + 3)
        xe[:, lo - (h0 - 3):hi - (h0 - 3), :] = x[b, :, lo:hi, :]
        xq = xe.reshape(C, NQT * 128).T                        # [2432, 128]
        xtp = np.ascontiguousarray(
            xq.reshape(NQT, 128, 128).transpose(1, 0, 2).reshape(128, NQT * 128)
        ).astype(BF16)
        in_maps.append({
            "xcm": xcm, "xtp": xtp, "w1sT": w1sT, "b1f": b1fc,
            "w2b": w2b, "idxt": idxt, "ident": ident,
        })
    return in_maps


def run(inputs: dict, trace: bool = False):
    from concourse.bass_utils import run_bass_kernel_spmd

    nc = _get_nc()
    in_maps = _host_prep(**inputs)
    res = run_bass_kernel_spmd(
        nc, in_maps, core_ids=list(range(8)), trace=trace,
    )
    out = np.zeros((B, C, H, W), dtype=np.float32)
    for core in range(8):
        b, half = divmod(core, 2)
        h0 = HH * half
        out[b, :, h0:h0 + HH, :] = (
            res.results[core]["out"].astype(np.float32).reshape(C, HH, W)
        )
    return out, res


def kernel(**inputs) -> np.ndarray:
    out, _ = run(inputs, trace=False)
    return out


# revision 7
# speedup vs baseline: 9.3897x; 1.2985x over previous
# Involution2d (K=7) Trainium2 kernel — 8-core SPMD, batch+spatial sharding.
#
# Sharding: 8 cores = (batch b in 0..3) x (H-half in 0..1); each core owns a
# [128, 32, 64] output block (2048 pixels, p = 64*h + w).
#
# Per-core algorithm (all-TensorE involution via a banded pixel->pixel matrix):
#   1. gen (bf16): 1x1 conv (BN folded) -> ReLU -> 1x1 conv, emitted directly
#      in pixel-major layout kermT[p, o] (16 matmuls of [33,128]^T @ [33,49];
#      bias rides an ones-row in the stationary operand).
#   2. GPSIMD local_scatter per 128-pixel tile mb: place the 49 kernel values
#      of pixel p at column q - 128*mb of A2T[p, :], where q = p + 192 +
#      64*di + dj is the flattened source pixel (38 rows x 64 cols q-space,
#      halo rows from the neighbor core, zeros at image edges). W-edge terms
#      get idx=-1 (skipped), which provably clips the window to 512 columns.
#   3. TensorE transposes each 128x128 block into per-q-tile strips
#      strip[kb] = A2[q, p-window] (bf16 transpose = 1 cyc/row).
#   4. involution: out[c, p] = sum_q xT[q, c] * A2[q, p] as 40 accumulating
#      bf16 matmuls over 512-column PSUM group tiles, kb-major so each xT
#      q-tile is loaded as stationary once.
import numpy as np
import ml_dtypes

EPS = 1e-5
KK = 7
C = 128
H = 64
W = 64
B = 4
HH = 32            # rows per core
P = HH * W         # 2048 output pixels per core
NQT = 19           # q tiles: (HH + 6) * W / 128
NO = 50            # offset count padded to even (49 + 1 dummy)
AWIN = 512         # scatter window (4 q-tiles)

_STATE = {}

BF16 = ml_dtypes.bfloat16


def _build():
    import concourse.tile as tile
    from concourse import bacc, mybir

    f32 = mybir.dt.float32
    bf16 = mybir.dt.bfloat16
    i16 = mybir.dt.int16
    nc = bacc.Bacc("TRN2", target_bir_lowering=False, debug=False)

    xcm_d = [
        nc.dram_tensor(f"xcm{i}", [C, P // 2], bf16, kind="ExternalInput").ap()
        for i in range(2)
    ]
    xtp_d = [
        nc.dram_tensor(f"xtp{i}", [128, n * 128], bf16, kind="ExternalInput").ap()
        for i, n in ((0, 10), (1, 9))
    ]
    w1sT_d = nc.dram_tensor("w1sT", [C, 32], bf16, kind="ExternalInput").ap()
    b1f_d = nc.dram_tensor("b1f", [32, 1], f32, kind="ExternalInput").ap()
    w2b_d = nc.dram_tensor("w2b", [33, 49], bf16, kind="ExternalInput").ap()
    ones_d = nc.dram_tensor("ones_row", [1, P], bf16, kind="ExternalInput").ap()
    idx_d = nc.dram_tensor("idxt", [128, NO], i16, kind="ExternalInput").ap()
    ident_d = nc.dram_tensor("ident", [128, 128], bf16, kind="ExternalInput").ap()
    out_d = nc.dram_tensor("out", [C, P], bf16, kind="ExternalOutput").ap()

    with tile.TileContext(nc) as tc:
        with (
            tc.tile_pool(name="consts", bufs=1) as cpool,
            tc.tile_pool(name="pgen", bufs=2, space="PSUM") as pgen,
            tc.tile_pool(name="pkt", bufs=1, space="PSUM") as pkt,
            tc.tile_pool(name="ptp", bufs=2, space="PSUM") as ptp,
            tc.tile_pool(name="pout", bufs=3, space="PSUM") as pout,
        ):
            # --- input DMAs on both HWDGE queues (sync + scalar) ---
            w1sT = cpool.tile([C, 32], bf16, tag="w1")
            nc.sync.dma_start(w1sT[:], w1sT_d)
            b1f = cpool.tile([32, 1], f32, tag="b1")
            nc.sync.dma_start(b1f[:], b1f_d)
            w2b = cpool.tile([33, 49], bf16, tag="w2")
            nc.sync.dma_start(w2b[:], w2b_d)
            fb = cpool.tile([33, P], bf16, tag="fb")
            nc.sync.dma_start(fb[32:33, :], ones_d)
            idxt = cpool.tile([128, NO], i16, tag="idx")
            nc.scalar.dma_start(idxt[:], idx_d)
            ident = cpool.tile([128, 128], bf16, tag="id")
            nc.scalar.dma_start(ident[:], ident_d)
            xcm = []
            for i in range(2):
                t = cpool.tile([C, P // 2], bf16, tag=f"xcm{i}")
                nc.sync.dma_start(t[:], xcm_d[i])
                xcm.append(t)
            xtp = []
            for i, n in ((0, 10), (1, 9)):
                t = cpool.tile([128, n * 128], bf16, tag=f"xtp{i}")
                nc.scalar.dma_start(t[:], xtp_d[i])
                xtp.append(t)

            def xtp_tile(kb):
                return (xtp[0][:, kb * 128:(kb + 1) * 128] if kb < 10
                        else xtp[1][:, (kb - 10) * 128:(kb - 9) * 128])

            outsb = cpool.tile([C, P], bf16, tag="outsb")

            # ---- kernel generation (pixel-major kermT[p, o]) ----
            GEN_CHUNK = 512
            for ci in range(4):
                sl = slice((ci % 2) * GEN_CHUNK, (ci % 2 + 1) * GEN_CHUNK)
                fsl = slice(ci * GEN_CHUNK, (ci + 1) * GEN_CHUNK)
                f1 = pgen.tile([32, GEN_CHUNK], f32, tag="f1")
                nc.tensor.matmul(f1[:], w1sT[:], xcm[ci // 2][:, sl],
                                 start=True, stop=True)
                nc.scalar.activation(
                    fb[0:32, fsl], f1[:], mybir.ActivationFunctionType.Relu,
                    bias=b1f[:],
                )
            kermT = []
            for t in range(16):
                kt = pkt.tile([128, 512], f32, tag="kt")
                nc.tensor.matmul(
                    kt[:, 0:49], fb[:, 128 * t:128 * (t + 1)], w2b[:],
                    start=True, stop=True,
                )
                km = cpool.tile([128, NO], bf16, tag=f"km{t}")
                if t % 2 == 0:
                    nc.vector.tensor_copy(km[:, 0:49], kt[:, 0:49])
                else:
                    nc.scalar.copy(km[:, 0:49], kt[:, 0:49])
                kermT.append(km)

            # ---- banded-matrix build (GPSIMD scatter) ----
            a2ts = []
            for mb in range(16):
                a2t = cpool.tile([128, AWIN], bf16, tag=f"a2t{mb}")
                nc.gpsimd.local_scatter(
                    a2t[:], kermT[mb][:], idxt[:],
                    channels=128, num_elems=AWIN, num_idxs=NO,
                )
                a2ts.append(a2t)

            # ---- transpose into strips + kb-major involution matmuls ----
            # strip[kb] holds A2[q-tile kb, p in [128(kb-3), 128(kb+1))).
            # group g accumulates psum po[g] = out[:, 512g:512(g+1)).
            po = {}
            for kb in range(NQT):
                mbs = [kb - j for j in range(3, -1, -1) if 0 <= kb - j <= 15]
                tp = ptp.tile([128, 1024], bf16, tag="tp")
                for mb in mbs:
                    pos = mb - (kb - 3)
                    nc.tensor.transpose(
                        tp[:, pos * 128:(pos + 1) * 128],
                        a2ts[mb][:, (kb - mb) * 128:(kb - mb + 1) * 128],
                        ident[:],
                    )
                strip = cpool.tile([128, AWIN], bf16, tag=f"st{kb % 4}")
                lo_pos = mbs[0] - (kb - 3)
                hi_pos = mbs[-1] - (kb - 3) + 1
                csl = slice(lo_pos * 128, hi_pos * 128)
                if kb % 2 == 0:
                    nc.vector.tensor_copy(strip[:, csl], tp[:, csl])
                else:
                    nc.scalar.copy(strip[:, csl], tp[:, csl])

                # matmuls: this q-tile contributes to groups g with
                # p-range [128(kb-3), 128(kb+1)) ∩ [512g, 512(g+1)).
                base = 128 * (kb - 3)
                for g in range(max(0, (kb - 3) // 4), min(3, kb // 4) + 1):
                    # group's first matmul (kb == 4g, start-slice only) uses
                    # start=True to clear the bank's has_written bits; every
                    # later matmul uses start=False — first touch of an
                    # element overwrites (bit clear), repeats accumulate.
                    first = g not in po
                    if first:
                        po[g] = pout.tile([C, 512], f32, name=f"po{g}", tag="po")
                    glo, ghi = 512 * g, 512 * (g + 1)
                    last = kb == min(4 * g + 6, NQT - 1)
                    # accumulate slice: cols [max(glo, base), 128*kb)
                    alo = max(glo, base)
                    ahi = min(ghi, 128 * kb)
                    slo = 128 * kb
                    shi = slo + 128
                    has_s = slo >= glo and shi <= ghi and kb <= 15
                    if ahi > alo:
                        nc.tensor.matmul(
                            po[g][:, alo - glo:ahi - glo],
                            xtp_tile(kb),
                            strip[:, alo - base:ahi - base],
                            start=False, stop=last and not has_s,
                            skip_group_check=True,
                        )
                    # start slice: cols [128kb, 128kb+128) if inside group
                    if has_s:
                        nc.tensor.matmul(
                            po[g][:, slo - glo:shi - glo],
                            xtp_tile(kb),
                            strip[:, slo - base:shi - base],
                            start=first, stop=last and has_s,
                            skip_group_check=True,
                        )
                    # group complete after kb == 4g + 6 (or last kb)
                    if last:
                        osl = slice(512 * g, 512 * (g + 1))
                        if g % 2 == 0:
                            nc.vector.tensor_copy(outsb[:, osl], po[g][:])
                        else:
                            nc.scalar.copy(outsb[:, osl], po[g][:])
                        nc.sync.dma_start(out_d[:, osl], outsb[:, osl])

    nc.compile()
    return nc


def _get_nc():
    if "nc" not in _STATE:
        _STATE["nc"] = _build()
    return _STATE["nc"]


def _make_idx_table():
    p_loc = np.arange(128)[:, None]
    o = np.arange(49)[None, :]
    di = o // 7 - 3
    dj = o % 7 - 3
    w_of = p_loc % 64
    idx = p_loc + 192 + 64 * di + dj
    masked = (w_of + dj < 0) | (w_of + dj >= 64)
    idx = np.where(masked, -1, idx)
    tab = np.full((128, NO), -1, dtype=np.int16)
    tab[:, :49] = idx.astype(np.int16)
    return tab


def _host_prep(x, w1, b1, bn_gamma, bn_beta, bn_mean, bn_var, w2, b2):
    x = np.asarray(x, dtype=np.float32)
    scale = np.asarray(bn_gamma) / np.sqrt(np.asarray(bn_var) + EPS)
    w1s = (np.asarray(w1) * scale[:, None]).astype(np.float32)
    b1f = (np.asarray(b1) * scale + np.asarray(bn_beta)
           - np.asarray(bn_mean) * scale).astype(np.float32)
    w1sT = np.ascontiguousarray(w1s.T).astype(BF16)            # [128, 32]
    w2b = np.vstack([np.asarray(w2, np.float32).T,
                     np.asarray(b2, np.float32)[None, :]]).astype(BF16)  # [33, 49]
    b1fc = np.ascontiguousarray(b1f[:, None])                  # [32, 1] f32
    ones_row = np.ones((1, P), dtype=np.float32).astype(BF16)
    idxt = _make_idx_table()                                   # [128, 50] i16
    ident = np.eye(128, dtype=np.float32).astype(BF16)

    in_maps = []
    for core in range(8):
        b, half = divmod(core, 2)
        h0 = HH * half
        xcm = np.ascontiguousarray(
            x[b, :, h0:h0 + HH, :].reshape(C, P)).astype(BF16)
        # q-space: rows h0-3 .. h0+35 (zeros outside the image)
        xe = np.zeros((C, HH + 6, W), dtype=np.float32)
        lo = max(0, h0 - 3)
        hi = min(H, h0 + HH + 3)
        xe[:, lo - (h0 - 3):hi - (h0 - 3), :] = x[b, :, lo:hi, :]
        xq = xe.reshape(C, NQT * 128).T                        # [2432, 128]
        xtp = np.ascontiguousarray(
            xq.reshape(NQT, 128, 128).transpose(1, 0, 2).reshape(128, NQT * 128)
        ).astype(BF16)
        in_maps.append({
            "xcm0": xcm[:, :P // 2], "xcm1": xcm[:, P // 2:],
            "xtp0": xtp[:, :10 * 128], "xtp1": xtp[:, 10 * 128:],
            "w1sT": w1sT, "b1f": b1fc, "w2b": w2b, "ones_row": ones_row,
            "idxt": idxt, "ident": ident,
        })
    return in_maps


def run(inputs: dict, trace: bool = False):
    from concourse.bass_utils import run_bass_kernel_spmd

    nc = _get_nc()
    in_maps = _host_prep(**inputs)
    res = run_bass_kernel_spmd(
        nc, in_maps, core_ids=list(range(8)), trace=trace,
    )
    out = np.zeros((B, C, H, W), dtype=np.float32)
    for core in range(8):
        b, half = divmod(core, 2)
        h0 = HH * half
        out[b, :, h0:h0 + HH, :] = (
            res.results[core]["out"].astype(np.float32).reshape(C, HH, W)
        )
    return out, res


def kernel(**inputs) -> np.ndarray:
    out, _ = run(inputs, trace=False)
    return out
